# revision 11
# baseline (speedup 1.0000x reference)
"""Trainium2 Bass kernel for KerasCrossAttentionModule (B=8, S=4096, D=256).

Sharding: data-parallel over batch B across 8 NeuronCores (1 batch/core).

Host prep (cheap, O(B*S*D)): fold the positional embeddings into q/k
(q + q_pos, k + k_pos), transpose to (D, S), cast fp16; rearrange v to a
key-major layout (128, 32*256) so the whole tensor is one SBUF tile.

Per-core device math (all engines pipelined):
    scoresT[j*128+p, i] = sum_d kh[d, jp] * qh[d, i]     (PE, fp16/fp32 acc)
    E = exp(scale * scoresT)                             (ACT, fp32 -> fp16)
    dacc[p, i] += E[jp, i]                               (DVE partial rowsum)
    av[d, i]  += sum_jp vr[jp, d] * E[jp, i]             (PE accum over j)
    denom = ones^T @ dacc   (M=128 -> full-width, no broadcast needed)
    out[d, i] = av[d, i] * (1 / denom[i])                (DVE approx recip)

The PE instruction stream is software-pipelined: the score matmuls for
iteration i+2 are issued before the AV matmuls of iteration i, so the
exp() latency never stalls the tensor engine.  Inputs stream over the
sync HWDGE queue and the gpsimd SWDGE queue in consumption order (the
ACT queue carries only activations - DMA triggers on it would delay the
first exp by ~20us).  A short warm-up matmul burst flips the PE HAM
clock-gate to full rate while the first input chunks land.

Output DRAM tensor is (D, S) == (DV, H*W), exactly the reference output
layout per batch, so no final transpose is needed.
"""

import os
import sys

import numpy as np

for _p in ("/opt/trn_rl_repo", "/root/.axon_site/_ro/trn_rl_repo"):
    if os.path.isdir(_p) and _p not in sys.path:
        sys.path.insert(0, _p)

import concourse.bass as bass
from concourse import bacc
import concourse.tile as tile
from concourse import mybir
from concourse.bass_utils import run_bass_kernel_spmd

B = 8
D = 256
S = 4096
HALF = 128            # partition-dim tile (D halves / key chunks)
N_DH = D // HALF      # 2 halves of the head dim
QSB = 1024            # query super-block (2 PSUM banks)
QC = 512              # matmul free-dim chunk (1 PSUM bank)
N_QC = QSB // QC
N_SB = S // QSB       # 4 query super-blocks
NJ = S // HALF        # 32 key chunks
SCALE = float(D) ** -0.5
N_WARM = 72           # PE warm-up matmuls (~5.6us: HAM flips warm at ~3.4us,
                      # the rest run 2x faster and bridge to DMA data-arrival)

FP32 = mybir.dt.float32
FP16 = mybir.dt.float16

# Set by test harness to capture a profile; harness-default is plain run.
TRACE = False
LAST_RESULT = None


def _build_attention():
    """One-core program; identical on all 8 cores (pure data parallel)."""
    nc = bacc.Bacc("TRN2")
    qh_d = nc.dram_tensor("qh", [D, S], FP16, kind="ExternalInput")
    kh_d = nc.dram_tensor("kh", [D, S], FP16, kind="ExternalInput")
    vr_d = nc.dram_tensor("vr", [HALF, NJ * D], FP16, kind="ExternalInput")
    out_d = nc.dram_tensor("out", [D, S], FP32, kind="ExternalOutput")

    with tile.TileContext(nc) as tc:
        with (
            tc.tile_pool(name="big", bufs=1) as big,
            tc.tile_pool(name="expp", bufs=8) as expp,
            tc.tile_pool(name="daccp", bufs=2) as daccp,
            tc.tile_pool(name="rbsp", bufs=2) as rbsp,
            tc.tile_pool(name="otp", bufs=4) as otp,
            tc.tile_pool(name="ps_s", bufs=2, space="PSUM") as ps_s,
            tc.tile_pool(name="ps_av", bufs=1, space="PSUM") as ps_av,
        ):
            qh = [big.tile([HALF, S], FP16, tag=f"qh{dh}", name=f"qh{dh}")
                  for dh in range(N_DH)]
            kh = [big.tile([HALF, S], FP16, tag=f"kh{dh}", name=f"kh{dh}")
                  for dh in range(N_DH)]
            vr = big.tile([HALF, NJ * D], FP16, tag="vr", name="vr")
            ones_h = big.tile([HALF, HALF], FP16, tag="ones_h", name="ones_h")
            nc.vector.memset(ones_h, 1.0)

            # ---- PE warm-up: flip the HAM clock gate to 8/8 while the
            # first input chunks stream in. ---------------------------------
            wt = ps_s.tile([HALF, QSB], FP32, tag="sp", name="warm")
            for _ in range(N_WARM):
                nc.tensor.matmul(wt[:, :HALF], ones_h, ones_h,
                                 start=True, stop=True)

            # ---- input DMAs, consumption order -----------------------------
            # The two queues (sync HWDGE + gpsimd SWDGE) run in parallel;
            # per-queue order matches first consumption by the main loop.
            # sync: k chunks (all consumed within the first sb); outputs are
            # appended later.
            for dh in range(N_DH):
                ds_ = slice(dh * HALF, (dh + 1) * HALF)
                nc.sync.dma_start(out=kh[dh][:, 0:512], in_=kh_d[ds_, 0:512])
            for dh in range(N_DH):
                ds_ = slice(dh * HALF, (dh + 1) * HALF)
                nc.sync.dma_start(out=qh[dh][:, 512:1024],
                                  in_=qh_d[ds_, 512:1024])
            for a, b in ((512, 1536), (1536, 2560), (2560, 3584),
                         (3584, 4096)):
                for dh in range(N_DH):
                    ds_ = slice(dh * HALF, (dh + 1) * HALF)
                    nc.sync.dma_start(out=kh[dh][:, a:b], in_=kh_d[ds_, a:b])
            # gpsimd: first half of the first q super-block, v chunks
            # (consumed from j=0), then the remaining q super-blocks.
            for dh in range(N_DH):
                ds_ = slice(dh * HALF, (dh + 1) * HALF)
                nc.gpsimd.dma_start(out=qh[dh][:, 0:512], in_=qh_d[ds_, 0:512])
            vchunks = [(0, 512), (512, 1024)] + [
                (1024 * i, 1024 * (i + 1)) for i in range(1, 8)
            ]
            for a, b in vchunks:
                nc.gpsimd.dma_start(out=vr[:, a:b], in_=vr_d[:, a:b])
            for sb in range(1, N_SB):
                for dh in range(N_DH):
                    ds_ = slice(dh * HALF, (dh + 1) * HALF)
                    cs = slice(sb * QSB, (sb + 1) * QSB)
                    nc.gpsimd.dma_start(out=qh[dh][:, cs], in_=qh_d[ds_, cs])

            # ---- software-pipelined main loop ------------------------------
            ets = {}
            daccs = {}
            avs = {}

            def do_s(i):
                sb, j = divmod(i, NJ)
                js = slice(j * HALF, (j + 1) * HALF)
                sp = ps_s.tile([HALF, QSB], FP32, tag="sp", name="sp")
                # dh outer so consecutive matmuls share the stationary
                # operand (one weight set per pair instead of per matmul).
                for dh in range(N_DH):
                    for c in range(N_QC):
                        cs = slice(sb * QSB + c * QC, sb * QSB + (c + 1) * QC)
                        nc.tensor.matmul(sp[:, c * QC:(c + 1) * QC],
                                         kh[dh][:, js], qh[dh][:, cs],
                                         start=(dh == 0), stop=(dh == 1))
                et = expp.tile([HALF, QSB], FP16, tag="et", name="et")
                nc.scalar.activation(
                    et, sp, mybir.ActivationFunctionType.Exp, scale=SCALE
                )
                if j == 0:
                    dacc = daccp.tile([HALF, QSB], FP16, tag="dacc",
                                      name="dacc")
                    nc.vector.tensor_copy(dacc, et)
                    daccs[sb] = dacc
                else:
                    nc.vector.tensor_add(daccs[sb], daccs[sb], et)
                ets[i] = et

            def do_av(i):
                sb, j = divmod(i, NJ)
                if j == 0:
                    avs[sb] = [
                        ps_av.tile([HALF, QSB], FP32, tag=f"av{dh}",
                                   name=f"av{dh}")
                        for dh in range(N_DH)
                    ]
                av = avs[sb]
                rbs = None
                et = ets.pop(i)
                for dh in range(N_DH):
                    vs = slice(j * D + dh * HALF, j * D + (dh + 1) * HALF)
                    for c in range(N_QC):
                        nc.tensor.matmul(av[dh][:, c * QC:(c + 1) * QC],
                                         vr[:, vs],
                                         et[:, c * QC:(c + 1) * QC],
                                         start=(j == 0), stop=(j == NJ - 1))
                    if j == NJ - 1 and dh == 0:
                        # Denominator reduce issued between the two AV halves:
                        # dacc is complete by now (its last add trails exp(j)
                        # by one pipeline stage), the dred wait hides behind
                        # the dh0 matmuls, and the DVE reciprocal overlaps the
                        # dh1 matmuls so av[]/rbs are both ready when the
                        # normalization muls start.
                        dacc = daccs.pop(sb)
                        dredt = ps_s.tile([HALF, QSB], FP32, tag="sp",
                                          name="dred")
                        for c in range(N_QC):
                            nc.tensor.matmul(dredt[:, c * QC:(c + 1) * QC],
                                             ones_h,
                                             dacc[:, c * QC:(c + 1) * QC],
                                             start=True, stop=True)
                        rbs = rbsp.tile([HALF, QSB], FP32, tag="rbs",
                                        name="rbs")
                        for c in range(N_QC):
                            cs = slice(c * QC, (c + 1) * QC)
                            nc.vector.reciprocal_approx_fast(out=rbs[:, cs],
                                                             in_=dredt[:, cs])
                if j == NJ - 1:
                    # Normalize + store in bank-sized chunks so the DVE muls,
                    # the output DMAs, and the next sb's AV matmuls pipeline.
                    # dh0 stores go out on sync (HWDGE), dh1 on the otherwise
                    # idle gpsimd queue, so the final stores trigger in
                    # parallel instead of queueing behind each other.
                    for dh in range(N_DH):
                        dma = nc.sync.dma_start if dh == 0 else \
                            nc.gpsimd.dma_start
                        ot = otp.tile([HALF, QSB], FP32, tag="ot", name="ot")
                        for c in range(N_QC):
                            cs = slice(c * QC, (c + 1) * QC)
                            nc.vector.tensor_mul(ot[:, cs], av[dh][:, cs],
                                                 rbs[:, cs])
                            dma(
                                out=out_d[dh * HALF:(dh + 1) * HALF,
                                          sb * QSB + c * QC:
                                          sb * QSB + (c + 1) * QC],
                                in_=ot[:, cs],
                            )
                    avs.pop(sb)

            n_it = N_SB * NJ
            do_s(0)
            do_s(1)
            for i in range(2, n_it):
                do_s(i)
                do_av(i - 2)
            do_av(n_it - 2)
            do_av(n_it - 1)
    nc.finalize()
    return nc


_NC_CACHE = {}


def _get_program():
    if "nc" not in _NC_CACHE:
        _NC_CACHE["nc"] = _build_attention()
    return _NC_CACHE["nc"]


def kernel(queries, keys, values, q_pos, k_pos):
    global LAST_RESULT
    q = np.asarray(queries, dtype=np.float32).reshape(B, D, S)
    k = np.asarray(keys, dtype=np.float32).reshape(B, D, S)
    v = np.asarray(values, dtype=np.float32).reshape(B, D, S)
    qpt = np.asarray(q_pos, np.float32).reshape(S, D).T       # (D, S)
    kpt = np.asarray(k_pos, np.float32).reshape(S, D).T
    qh = (q + qpt[None]).astype(np.float16)                   # (B, D, S)
    kh = (k + kpt[None]).astype(np.float16)
    # v (B, D, S) -> (B, 128, NJ*D): vr[b, p, j*D + d] = v[b, d, j*128 + p]
    vr = np.ascontiguousarray(
        v.reshape(B, D, NJ, HALF).transpose(0, 3, 2, 1).reshape(B, HALF, NJ * D)
    ).astype(np.float16)

    nc = _get_program()
    in_maps = [
        {
            "qh": np.ascontiguousarray(qh[b]),
            "kh": np.ascontiguousarray(kh[b]),
            "vr": vr[b],
        }
        for b in range(B)
    ]
    res = run_bass_kernel_spmd(nc, in_maps, list(range(B)), trace=TRACE)
    LAST_RESULT = res
    out = np.stack([res.results[b]["out"] for b in range(B)])  # (B, D, S)
    return out.reshape(B, D, 64, 64).astype(np.float32)


# revision 16
# speedup vs baseline: 1.0045x; 1.0045x over previous
"""Trainium2 Bass kernel for KerasCrossAttentionModule (B=8, S=4096, D=256).

Sharding: data-parallel over batch B across 8 NeuronCores (1 batch/core).

Host prep (cheap, O(B*S*D)): fold the positional embeddings into q/k
(q + q_pos, k + k_pos), transpose to (D, S), cast fp16; rearrange v to a
key-major layout (128, 32*256) so the whole tensor is one SBUF tile.

Per-core device math (all engines pipelined):
    scoresT[j*128+p, i] = sum_d kh[d, jp] * qh[d, i]     (PE, fp16/fp32 acc)
    E = exp(scale * scoresT)                             (ACT, fp32 -> fp16)
    dacc[p, i] += E[jp, i]                               (DVE partial rowsum)
    av[d, i]  += sum_jp vr[jp, d] * E[jp, i]             (PE accum over j)
    denom = ones^T @ dacc   (M=128 -> full-width, no broadcast needed)
    out[d, i] = av[d, i] * (1 / denom[i])                (DVE approx recip)

The PE instruction stream is software-pipelined: the score matmuls for
iteration i+2 are issued before the AV matmuls of iteration i, so the
exp() latency never stalls the tensor engine.  Inputs stream over the
sync HWDGE queue and the gpsimd SWDGE queue in consumption order (the
ACT queue carries only activations - DMA triggers on it would delay the
first exp by ~20us).  A short warm-up matmul burst flips the PE HAM
clock-gate to full rate while the first input chunks land.

Output DRAM tensor is (D, S) == (DV, H*W), exactly the reference output
layout per batch, so no final transpose is needed.
"""

import os
import sys

import numpy as np

for _p in ("/opt/trn_rl_repo", "/root/.axon_site/_ro/trn_rl_repo"):
    if os.path.isdir(_p) and _p not in sys.path:
        sys.path.insert(0, _p)

import concourse.bass as bass
from concourse import bacc
import concourse.tile as tile
from concourse import mybir
from concourse.bass_utils import run_bass_kernel_spmd

B = 8
D = 256
S = 4096
HALF = 128            # partition-dim tile (D halves / key chunks)
N_DH = D // HALF      # 2 halves of the head dim
QSB = 1024            # query super-block (2 PSUM banks)
QC = 512              # matmul free-dim chunk (1 PSUM bank)
N_QC = QSB // QC
N_SB = S // QSB       # 4 query super-blocks
NJ = S // HALF        # 32 key chunks
SCALE = float(D) ** -0.5
N_WARM = 72           # PE warm-up matmuls (~5.6us: HAM flips warm at ~3.4us,
                      # the rest run 2x faster and bridge to DMA data-arrival)

FP32 = mybir.dt.float32
FP16 = mybir.dt.float16

# Set by test harness to capture a profile; harness-default is plain run.
TRACE = False
LAST_RESULT = None


def _build_attention():
    """One-core program; identical on all 8 cores (pure data parallel)."""
    nc = bacc.Bacc("TRN2")
    qh_d = nc.dram_tensor("qh", [D, S], FP16, kind="ExternalInput")
    kh_d = nc.dram_tensor("kh", [D, S], FP16, kind="ExternalInput")
    vr_d = nc.dram_tensor("vr", [HALF, NJ * D], FP16, kind="ExternalInput")
    out_d = nc.dram_tensor("out", [D, S], FP32, kind="ExternalOutput")

    with tile.TileContext(nc) as tc:
        with (
            tc.tile_pool(name="big", bufs=1) as big,
            tc.tile_pool(name="expp", bufs=8) as expp,
            tc.tile_pool(name="daccp", bufs=2) as daccp,
            tc.tile_pool(name="rbsp", bufs=2) as rbsp,
            tc.tile_pool(name="otp", bufs=4) as otp,
            tc.tile_pool(name="ps_s", bufs=2, space="PSUM") as ps_s,
            tc.tile_pool(name="ps_av", bufs=1, space="PSUM") as ps_av,
        ):
            qh = [big.tile([HALF, S], FP16, tag=f"qh{dh}", name=f"qh{dh}")
                  for dh in range(N_DH)]
            kh = [big.tile([HALF, S], FP16, tag=f"kh{dh}", name=f"kh{dh}")
                  for dh in range(N_DH)]
            vr = big.tile([HALF, NJ * D], FP16, tag="vr", name="vr")
            ones_h = big.tile([HALF, HALF], FP16, tag="ones_h", name="ones_h")
            nc.vector.memset(ones_h, 1.0)

            # ---- PE warm-up: flip the HAM clock gate to 8/8 while the
            # first input chunks stream in. ---------------------------------
            wt = ps_s.tile([HALF, QSB], FP32, tag="sp", name="warm")
            for _ in range(N_WARM):
                nc.tensor.matmul(wt[:, :HALF], ones_h, ones_h,
                                 start=True, stop=True)

            # ---- input DMAs, consumption order -----------------------------
            # The two queues (sync HWDGE + gpsimd SWDGE) run in parallel;
            # per-queue order matches first consumption by the main loop.
            # sync: k chunks (all consumed within the first sb); outputs are
            # appended later.
            for dh in range(N_DH):
                ds_ = slice(dh * HALF, (dh + 1) * HALF)
                nc.sync.dma_start(out=kh[dh][:, 0:512], in_=kh_d[ds_, 0:512])
            for dh in range(N_DH):
                ds_ = slice(dh * HALF, (dh + 1) * HALF)
                nc.sync.dma_start(out=qh[dh][:, 512:1024],
                                  in_=qh_d[ds_, 512:1024])
            for a, b in ((512, 1536), (1536, 2560), (2560, 3584),
                         (3584, 4096)):
                for dh in range(N_DH):
                    ds_ = slice(dh * HALF, (dh + 1) * HALF)
                    nc.sync.dma_start(out=kh[dh][:, a:b], in_=kh_d[ds_, a:b])
            # gpsimd: first half of the first q super-block, v chunks
            # (consumed from j=0), then the remaining q super-blocks.
            for dh in range(N_DH):
                ds_ = slice(dh * HALF, (dh + 1) * HALF)
                nc.gpsimd.dma_start(out=qh[dh][:, 0:512], in_=qh_d[ds_, 0:512])
            vchunks = [(0, 512), (512, 1024)] + [
                (1024 * i, 1024 * (i + 1)) for i in range(1, 8)
            ]
            for a, b in vchunks:
                nc.gpsimd.dma_start(out=vr[:, a:b], in_=vr_d[:, a:b])
            for sb in range(1, N_SB):
                for dh in range(N_DH):
                    ds_ = slice(dh * HALF, (dh + 1) * HALF)
                    cs = slice(sb * QSB, (sb + 1) * QSB)
                    nc.gpsimd.dma_start(out=qh[dh][:, cs], in_=qh_d[ds_, cs])

            # ---- software-pipelined main loop ------------------------------
            ets = {}
            daccs = {}
            avs = {}

            def do_s(i):
                sb, j = divmod(i, NJ)
                js = slice(j * HALF, (j + 1) * HALF)
                sp = ps_s.tile([HALF, QSB], FP32, tag="sp", name="sp")
                for c in range(N_QC):
                    cs = slice(sb * QSB + c * QC, sb * QSB + (c + 1) * QC)
                    for dh in range(N_DH):
                        nc.tensor.matmul(sp[:, c * QC:(c + 1) * QC],
                                         kh[dh][:, js], qh[dh][:, cs],
                                         start=(dh == 0), stop=(dh == 1))
                et = expp.tile([HALF, QSB], FP16, tag="et", name="et")
                nc.scalar.activation(
                    et, sp, mybir.ActivationFunctionType.Exp, scale=SCALE
                )
                if j == 0:
                    dacc = daccp.tile([HALF, QSB], FP16, tag="dacc",
                                      name="dacc")
                    nc.vector.tensor_copy(dacc, et)
                    daccs[sb] = dacc
                else:
                    nc.vector.tensor_add(daccs[sb], daccs[sb], et)
                ets[i] = et

            def do_av(i):
                sb, j = divmod(i, NJ)
                if j == 0:
                    avs[sb] = [
                        ps_av.tile([HALF, QSB], FP32, tag=f"av{dh}",
                                   name=f"av{dh}")
                        for dh in range(N_DH)
                    ]
                av = avs[sb]
                rbs = None
                if j == NJ - 1:
                    # Denominator reduce + reciprocal issued BEFORE the last
                    # AV pair: dacc is already complete (its last add trails
                    # exp(j) by one pipeline stage), so the DVE reciprocal
                    # overlaps the final AV matmuls and av[]/rbs are both
                    # ready when the normalization muls start.
                    dacc = daccs.pop(sb)
                    dredt = ps_s.tile([HALF, QSB], FP32, tag="sp",
                                      name="dred")
                    for c in range(N_QC):
                        nc.tensor.matmul(dredt[:, c * QC:(c + 1) * QC],
                                         ones_h,
                                         dacc[:, c * QC:(c + 1) * QC],
                                         start=True, stop=True)
                    rbs = rbsp.tile([HALF, QSB], FP32, tag="rbs", name="rbs")
                    for c in range(N_QC):
                        cs = slice(c * QC, (c + 1) * QC)
                        nc.vector.reciprocal_approx_fast(out=rbs[:, cs],
                                                         in_=dredt[:, cs])
                et = ets.pop(i)
                for dh in range(N_DH):
                    vs = slice(j * D + dh * HALF, j * D + (dh + 1) * HALF)
                    for c in range(N_QC):
                        nc.tensor.matmul(av[dh][:, c * QC:(c + 1) * QC],
                                         vr[:, vs],
                                         et[:, c * QC:(c + 1) * QC],
                                         start=(j == 0), stop=(j == NJ - 1))
                if j == NJ - 1:
                    # Normalize + store in bank-sized chunks so the DVE muls,
                    # the output DMAs, and the next sb's AV matmuls pipeline.
                    # dh0 stores go out on sync (HWDGE), dh1 on the otherwise
                    # idle gpsimd queue, so the final stores trigger in
                    # parallel instead of queueing behind each other.
                    for dh in range(N_DH):
                        dma = nc.sync.dma_start if dh == 0 else \
                            nc.gpsimd.dma_start
                        ot = otp.tile([HALF, QSB], FP32, tag="ot", name="ot")
                        for c in range(N_QC):
                            cs = slice(c * QC, (c + 1) * QC)
                            nc.vector.tensor_mul(ot[:, cs], av[dh][:, cs],
                                                 rbs[:, cs])
                            dma(
                                out=out_d[dh * HALF:(dh + 1) * HALF,
                                          sb * QSB + c * QC:
                                          sb * QSB + (c + 1) * QC],
                                in_=ot[:, cs],
                            )
                    avs.pop(sb)

            n_it = N_SB * NJ
            do_s(0)
            do_s(1)
            for i in range(2, n_it):
                do_s(i)
                do_av(i - 2)
            do_av(n_it - 2)
            do_av(n_it - 1)
    nc.finalize()
    return nc


_NC_CACHE = {}


def _get_program():
    if "nc" not in _NC_CACHE:
        _NC_CACHE["nc"] = _build_attention()
    return _NC_CACHE["nc"]


def kernel(queries, keys, values, q_pos, k_pos):
    global LAST_RESULT
    q = np.asarray(queries, dtype=np.float32).reshape(B, D, S)
    k = np.asarray(keys, dtype=np.float32).reshape(B, D, S)
    v = np.asarray(values, dtype=np.float32).reshape(B, D, S)
    qpt = np.asarray(q_pos, np.float32).reshape(S, D).T       # (D, S)
    kpt = np.asarray(k_pos, np.float32).reshape(S, D).T
    qh = (q + qpt[None]).astype(np.float16)                   # (B, D, S)
    kh = (k + kpt[None]).astype(np.float16)
    # v (B, D, S) -> (B, 128, NJ*D): vr[b, p, j*D + d] = v[b, d, j*128 + p]
    vr = np.ascontiguousarray(
        v.reshape(B, D, NJ, HALF).transpose(0, 3, 2, 1).reshape(B, HALF, NJ * D)
    ).astype(np.float16)

    nc = _get_program()
    in_maps = [
        {
            "qh": np.ascontiguousarray(qh[b]),
            "kh": np.ascontiguousarray(kh[b]),
            "vr": vr[b],
        }
        for b in range(B)
    ]
    res = run_bass_kernel_spmd(nc, in_maps, list(range(B)), trace=TRACE)
    LAST_RESULT = res
    out = np.stack([res.results[b]["out"] for b in range(B)])  # (B, D, S)
    return out.reshape(B, D, 64, 64).astype(np.float32)
